# revision 9
# baseline (speedup 1.0000x reference)
"""Trainium2 Bass kernel for a 2-layer relational GNN (ConceptGNN).

Math per layer (reference):
    msg  = x[src] * rel_emb[edge_attr]               # [E, D]
    agg  = segment_sum(msg, dst) / max(deg, 1)       # [N, D] mean
    h    = relu((x + agg) @ W)

Distribution: destination-sharded across 8 NeuronCores. Core k owns dst
nodes [k*6250, (k+1)*6250), padded to a 6400-row shard (25 blocks of 256
nodes). The full padded node table (51200 rows) is replicated so each
core can gather arbitrary source rows. Gathers use the DMAGatherAnt
ucode (int16 indices), so the table is split into two 25600-row halves
and each (block, rel) edge cell is sub-grouped by source half.

Aggregation runs on the TensorEngine as a one-hot matmul: for each
128-edge chunk, the VectorEngine builds S[e, n] = (n == dst_rel_e) * w_e
(w_e = 1/max(deg,1), so the matmul yields the mean directly) and PE
accumulates aggT_r += msg_chunk.T @ S_chunk into PSUM, per relation r.
Since (m * rel_r) @ W == m @ (diag(rel_r) @ W), the per-edge gating by
rel_emb folds into 6 pre-scaled weight matrices W_r, so messages never
need a per-edge elementwise multiply:
    h.T = relu(W.T @ x.T + sum_r W_r.T @ aggT_r)
Matmuls use float32r (fp32 replication mode, ~1.5e-4 rel rounding) with
a 256-wide moving dim for 1 cycle/row.

The same compiled program runs twice: layer 1 from x, layer 2 from the
host-assembled h1 table. Edges are identical across layers, so the chunk
structure (compile-time constant) is shared.
"""

import numpy as np

# ---- problem constants (fixed by the harness contract) ----
N_NODES = 50000
N_EDGES = 640000
D = 128
N_REL = 6
NCORES = 8
P = 128

OWN = N_NODES // NCORES  # 6250 real nodes per core
BLK = 256                # node block width (matmul moving dim; >=256 for f32r)
SHARD = 6400             # padded shard rows; SHARD % BLK == 0
NBLK = SHARD // BLK      # 25


def _build_layer_nc(chunksA, chunksB, *, d=D, n_rel=N_REL, shard=SHARD,
                    blk=BLK, ncores=NCORES):
    """Build the single-layer Bass program.

    chunksA/chunksB: [nblk][n_rel] 128-edge chunk counts per (block, rel)
    for source rows in the low/high table half — identical across cores.
    """
    import concourse.bass as bass
    import concourse.bacc as bacc
    import concourse.mybir as mybir
    import concourse.tile as tile

    f32 = mybir.dt.float32
    f32r = mybir.dt.float32r
    i16 = mybir.dt.int16
    Act = mybir.ActivationFunctionType
    Alu = mybir.AluOpType

    nblk = shard // blk
    nh = blk // P
    half_rows = ncores * shard // 2
    assert half_rows < 32768
    chunksA = [[int(c) for c in row] for row in chunksA]
    chunksB = [[int(c) for c in row] for row in chunksB]
    GA = sum(sum(r) for r in chunksA)
    GB = sum(sum(r) for r in chunksB)
    G = GA + GB
    maxcbA = max(sum(r) for r in chunksA)
    maxcbB = max(sum(r) for r in chunksB)

    nc = bacc.Bacc("TRN2", target_bir_lowering=False, debug=False,
                   num_devices=ncores)
    xtabA = nc.dram_tensor("xtabA", [half_rows, d], f32r, kind="ExternalInput")
    xtabB = nc.dram_tensor("xtabB", [half_rows, d], f32r, kind="ExternalInput")
    xown = nc.dram_tensor("xown", [shard, d], f32, kind="ExternalInput")
    wm = nc.dram_tensor("wm", [n_rel + 1, d, d], f32r, kind="ExternalInput")
    gidxA = nc.dram_tensor("gidxA", [P, max(GA, 1) * 8], i16,
                           kind="ExternalInput")
    gidxB = nc.dram_tensor("gidxB", [P, max(GB, 1) * 8], i16,
                           kind="ExternalInput")
    drel = nc.dram_tensor("drel", [P, G], f32, kind="ExternalInput")
    wts = nc.dram_tensor("wts", [P, G], f32, kind="ExternalInput")
    iota = nc.dram_tensor("iota", [P, blk], f32, kind="ExternalInput")
    ident = nc.dram_tensor("ident", [P, P], f32, kind="ExternalInput")
    hout = nc.dram_tensor("hout", [shard, d], f32, kind="ExternalOutput")

    with tile.TileContext(nc) as tc:
        with (
            tc.tile_pool(name="const", bufs=1) as cpool,
            tc.tile_pool(name="gatherA", bufs=3) as gpoolA,
            tc.tile_pool(name="gatherB", bufs=3) as gpoolB,
            tc.tile_pool(name="meta", bufs=3) as mpool,
            tc.tile_pool(name="onehot", bufs=4) as spool,
            tc.tile_pool(name="agg", bufs=2) as apool,
            tc.tile_pool(name="hwork", bufs=2) as hpool,
            tc.tile_pool(name="psA", bufs=2, space="PSUM") as psA,
            tc.tile_pool(name="psH", bufs=2, space="PSUM") as psH,
            tc.tile_pool(name="psT", bufs=2, space="PSUM") as psT,
        ):
            iot = cpool.tile([P, blk], f32, tag="iota")
            nc.sync.dma_start(iot[:], iota[:])
            idn = cpool.tile([P, P], f32, tag="ident")
            nc.sync.dma_start(idn[:], ident[:])
            wmt = cpool.tile([P, (n_rel + 1) * d], f32r, tag="wm")
            for j in range(n_rel + 1):
                nc.sync.dma_start(wmt[:, j * d:(j + 1) * d], wm[j])

            col = 0        # global meta column (emission order)
            colA = 0       # global A-region gather column
            colB = 0
            for b in range(nblk):
                cbA = sum(chunksA[b])
                cbB = sum(chunksB[b])
                cb = cbA + cbB
                # gathered source rows for this block, per table half
                if cbA:
                    xgA = gpoolA.tile([P, maxcbA, d], f32r, tag="xgA")
                    ixA = mpool.tile([P, max(maxcbA, 1) * 8], i16, tag="ixA")
                    nc.sync.dma_start(
                        ixA[:, :cbA * 8], gidxA[:, colA * 8:(colA + cbA) * 8])
                    for g0 in range(0, cbA, 8):
                        gn = min(8, cbA - g0)
                        nc.gpsimd.dma_gather(
                            out_ap=xgA[:, g0:g0 + gn, :], in_ap=xtabA[:],
                            idxs_ap=ixA[:, g0 * 8:(g0 + gn) * 8],
                            num_idxs=gn * P, num_idxs_reg=gn * P,
                            elem_size=d)
                if cbB:
                    xgB = gpoolB.tile([P, maxcbB, d], f32r, tag="xgB")
                    ixB = mpool.tile([P, max(maxcbB, 1) * 8], i16, tag="ixB")
                    nc.sync.dma_start(
                        ixB[:, :cbB * 8], gidxB[:, colB * 8:(colB + cbB) * 8])
                    for g0 in range(0, cbB, 8):
                        gn = min(8, cbB - g0)
                        nc.gpsimd.dma_gather(
                            out_ap=xgB[:, g0:g0 + gn, :], in_ap=xtabB[:],
                            idxs_ap=ixB[:, g0 * 8:(g0 + gn) * 8],
                            num_idxs=gn * P, num_idxs_reg=gn * P,
                            elem_size=d)
                dr = mpool.tile([P, max(cb, 1)], f32, tag="dr")
                wt = mpool.tile([P, max(cb, 1)], f32, tag="wt")
                nc.sync.dma_start(dr[:, :cb], drel[:, col:col + cb])
                nc.sync.dma_start(wt[:, :cb], wts[:, col:col + cb])

                aggT = apool.tile([P, n_rel, blk], f32r, tag="aggT")
                ci = 0      # block-local meta column, emission order
                ciA = 0     # block-local gather columns
                ciB = 0
                for r in range(n_rel):
                    nA, nB = chunksA[b][r], chunksB[b][r]
                    ntot = nA + nB
                    ps = psA.tile([P, blk], f32, tag="psA")
                    for c in range(ntot):
                        if c < nA:
                            lhs = xgA[:, ciA, :]
                            ciA += 1
                        else:
                            lhs = xgB[:, ciB, :]
                            ciB += 1
                        S = spool.tile([P, blk], f32r, tag="S")
                        nc.vector.tensor_scalar(
                            out=S[:], in0=iot[:],
                            scalar1=dr[:, ci:ci + 1],
                            scalar2=wt[:, ci:ci + 1],
                            op0=Alu.is_equal, op1=Alu.mult,
                        )
                        nc.tensor.matmul(
                            ps[:], lhsT=lhs, rhs=S[:],
                            start=(c == 0), stop=(c == ntot - 1),
                        )
                        ci += 1
                    nc.scalar.activation(aggT[:, r, :], ps[:], Act.Copy)

                # dense phase: h.T = relu(W.T x.T + sum_r W_r.T aggT_r)
                xr = hpool.tile([P, nh, d], f32, tag="xr")
                for h in range(nh):
                    nc.sync.dma_start(
                        xr[:, h, :],
                        xown[b * blk + h * P: b * blk + (h + 1) * P, :])
                pxT = psT.tile([P, blk], f32, tag="psT")
                for h in range(nh):
                    nc.tensor.transpose(
                        pxT[:, h * P:(h + 1) * P], xr[:, h, :], idn[:])
                xT = hpool.tile([P, blk], f32r, tag="xT")
                nc.vector.tensor_copy(xT[:], pxT[:])
                ph = psH.tile([P, blk], f32, tag="psH")
                nc.tensor.matmul(
                    ph[:], lhsT=wmt[:, 0:d],
                    rhs=xT[:], start=True, stop=False)
                for r in range(n_rel):
                    nc.tensor.matmul(
                        ph[:],
                        lhsT=wmt[:, (1 + r) * d:(2 + r) * d],
                        rhs=aggT[:, r, :],
                        start=False, stop=(r == n_rel - 1))
                hT = hpool.tile([P, blk], f32, tag="hT")
                nc.scalar.activation(hT[:], ph[:], Act.Relu)
                for h in range(nh):
                    pr = psT.tile([P, P], f32, tag="psT")
                    nc.tensor.transpose(
                        pr[:], hT[:, h * P:(h + 1) * P], idn[:])
                    hr = hpool.tile([P, P], f32, tag="hr")
                    nc.vector.tensor_copy(hr[:], pr[:])
                    nc.sync.dma_start(
                        hout[b * blk + h * P: b * blk + (h + 1) * P, :],
                        hr[:])
                col += cb
                colA += cbA
                colB += cbB
    nc.compile()
    return nc


def _preprocess(edge_index, edge_attr, *, n_nodes=N_NODES, n_rel=N_REL,
                own=OWN, shard=SHARD, blk=BLK, ncores=NCORES):
    """Index-only host preprocessing.

    Edges are assigned to (core, block, rel, half) cells (half = which
    25600-row table half the remapped source row falls in), padded to
    whole 128-edge chunks with per-cell counts equalized across cores.

    Returns (chunksA, chunksB, gidxA16, gidxB16, drelT, wT):
      gidx*16: [ncores, 128, G* * 8] int16 in DMAGatherAnt wrap layout
               (16-partition wrap, replicated 8x down the partitions).
      drelT/wT: [ncores, 128, G] with edge (p, g) = linear g*128 + p in
               emission order (per block: r0[A then B], r1[A then B]...).
    """
    ei = np.asarray(edge_index)
    src = ei[0].astype(np.int64)
    dst = ei[1].astype(np.int64)
    attr = np.asarray(edge_attr).astype(np.int64)
    nblk = shard // blk
    half_rows = ncores * shard // 2

    deg = np.bincount(dst, minlength=n_nodes)
    w_e = (1.0 / np.maximum(deg, 1.0)).astype(np.float32)[dst]
    rsrc = ((src // own) * shard + (src % own)).astype(np.int64)
    half = (rsrc >= half_rows).astype(np.int64)
    core = dst // own
    dloc = dst - core * own
    b_e = dloc // blk
    drel_e = (dloc - b_e * blk).astype(np.float32)

    # cell id ordered (core, block, rel, half)
    gid = (((core * nblk) + b_e) * n_rel + attr) * 2 + half
    order = np.argsort(gid, kind="stable")
    ncell = nblk * n_rel * 2
    counts = np.bincount(gid, minlength=ncores * ncell).reshape(ncores, ncell)
    chunks_flat = (-(-counts // P)).max(axis=0)  # [ncell], per-cell max
    cA = chunks_flat[0::2].reshape(nblk, n_rel).copy()
    cB = chunks_flat[1::2].reshape(nblk, n_rel).copy()
    empty = (cA + cB) == 0
    cA[empty] = 1
    chunks_flat = np.stack([cA.reshape(-1), cB.reshape(-1)], axis=1).reshape(-1)

    GA = int(cA.sum())
    GB = int(cB.sum())
    G = GA + GB

    # meta layout (emission order): per block, r0A,r0B,r1A,r1B,...
    meta_off = np.zeros(ncell + 1, np.int64)
    meta_off[1:] = np.cumsum(chunks_flat) * P
    # gather layouts (region order): per block, all A cells r0..r5 then next
    # block; likewise for B. Compute per-cell offsets into A/B index arrays.
    offA = np.zeros((nblk, n_rel), np.int64)
    offB = np.zeros((nblk, n_rel), np.int64)
    accA = accB = 0
    for b in range(nblk):
        for r in range(n_rel):
            offA[b, r] = accA
            accA += cA[b, r]
            offB[b, r] = accB
            accB += cB[b, r]

    gidxA16 = np.zeros((ncores, GA * P), np.int16)
    gidxB16 = np.zeros((ncores, GB * P), np.int16)
    drelA = np.zeros((ncores, G * P), np.float32)
    wA = np.zeros((ncores, G * P), np.float32)

    srt_rsrc = rsrc[order]
    srt_drel = drel_e[order]
    srt_w = w_e[order]
    gstart = np.zeros(ncores * ncell + 1, np.int64)
    gstart[1:] = np.cumsum(counts.reshape(-1))
    for k in range(ncores):
        for cell in range(ncell):
            g = k * ncell + cell
            s, e = gstart[g], gstart[g + 1]
            n = e - s
            if n == 0:
                continue
            o = meta_off[cell]
            drelA[k, o:o + n] = srt_drel[s:e]
            wA[k, o:o + n] = srt_w[s:e]
            blockrel = cell // 2
            b, r = blockrel // n_rel, blockrel % n_rel
            if cell % 2 == 0:
                oi = offA[b, r] * P
                gidxA16[k, oi:oi + n] = srt_rsrc[s:e].astype(np.int16)
            else:
                oi = offB[b, r] * P
                gidxB16[k, oi:oi + n] = (srt_rsrc[s:e]
                                         - half_rows).astype(np.int16)

    # meta arrays to [P, G]: edge (p, g) = linear g*P + p
    drelT = np.ascontiguousarray(drelA.reshape(ncores, G, P).transpose(0, 2, 1))
    wT = np.ascontiguousarray(wA.reshape(ncores, G, P).transpose(0, 2, 1))

    # index arrays to DMAGatherAnt wrap: [16, num/16] with
    # idx16[p, s] = idx[s*16 + p], then replicated 8x to 128 partitions.
    def wrap(a, Gn):
        if Gn == 0:
            return np.zeros((ncores, P, 8), np.int16)
        a = a.reshape(ncores, Gn * P // 16, 16).transpose(0, 2, 1)
        return np.ascontiguousarray(np.tile(a, (1, 8, 1)))

    return (cA.tolist(), cB.tolist(), wrap(gidxA16, GA), wrap(gidxB16, GB),
            drelT, wT)


_COMPILED = {}


def _get_nc(chunksA, chunksB):
    key = (tuple(tuple(r) for r in chunksA), tuple(tuple(r) for r in chunksB))
    if key not in _COMPILED:
        _COMPILED[key] = _build_layer_nc(chunksA, chunksB)
    return _COMPILED[key]


def _wmats(W, rel_emb):
    # W_r = diag(rel_r) @ W so that (m * rel_r) @ W == m @ W_r
    return np.ascontiguousarray(
        np.concatenate([W[None], rel_emb[:, :, None] * W[None]], axis=0)
    ).astype(np.float32)


def kernel(x, rel_emb, W1, W2, edge_index, edge_attr, _trace=False):
    from concourse.bass_utils import run_bass_kernel_spmd

    x = np.ascontiguousarray(np.asarray(x, np.float32))
    rel_emb = np.ascontiguousarray(np.asarray(rel_emb, np.float32))
    W1 = np.ascontiguousarray(np.asarray(W1, np.float32))
    W2 = np.ascontiguousarray(np.asarray(W2, np.float32))

    chunksA, chunksB, gA16, gB16, drelT, wT = _preprocess(
        edge_index, edge_attr)
    nc = _get_nc(chunksA, chunksB)
    iota = np.ascontiguousarray(
        np.broadcast_to(np.arange(BLK, dtype=np.float32), (P, BLK)))
    ident = np.eye(P, dtype=np.float32)
    half_rows = NCORES * SHARD // 2

    results = []

    def run_layer(xtab, wmats):
        xtab_v = xtab.reshape(NCORES, SHARD, D)
        in_maps = [
            dict(xtabA=xtab[:half_rows], xtabB=xtab[half_rows:],
                 xown=np.ascontiguousarray(xtab_v[k]), wm=wmats,
                 gidxA=gA16[k], gidxB=gB16[k], drel=drelT[k], wts=wT[k],
                 iota=iota, ident=ident)
            for k in range(NCORES)
        ]
        res = run_bass_kernel_spmd(
            nc, in_maps, core_ids=list(range(NCORES)), trace=False)
        results.append(res)
        return np.concatenate([r["hout"] for r in res.results], axis=0)

    wm1 = _wmats(W1, rel_emb)
    wm2 = _wmats(W2, rel_emb)
    xtab1 = np.zeros((NCORES * SHARD, D), np.float32)
    xtab1.reshape(NCORES, SHARD, D)[:, :OWN] = x.reshape(NCORES, OWN, D)
    h1 = run_layer(xtab1, wm1)
    h2 = run_layer(np.ascontiguousarray(h1), wm2)
    out = np.ascontiguousarray(
        h2.reshape(NCORES, SHARD, D)[:, :OWN].reshape(N_NODES, D))
    if _trace:
        kernel._last_results = results
    return out


# revision 11
# speedup vs baseline: 25837.1087x; 25837.1087x over previous
"""Trainium2 Bass kernel for a 2-layer relational GNN (ConceptGNN).

Math per layer (reference):
    msg  = x[src] * rel_emb[edge_attr]               # [E, D]
    agg  = segment_sum(msg, dst) / max(deg, 1)       # [N, D] mean
    h    = relu((x + agg) @ W)

Distribution: destination-sharded across 8 NeuronCores. Core k owns dst
nodes [k*6250, (k+1)*6250), padded to a 6400-row shard (25 blocks of 256
nodes). The full padded node table (51200 rows) is replicated so each
core can gather arbitrary source rows. Gathers use the DMAGatherAnt
ucode (int16 indices), so the table is split into two 25600-row halves
and each (block, rel) edge cell is sub-grouped by source half.

Aggregation runs on the TensorEngine as a one-hot matmul: for each
128-edge chunk, the VectorEngine builds S[e, n] = (n == dst_rel_e) * w_e
(w_e = 1/max(deg,1), so the matmul yields the mean directly) and PE
accumulates aggT_r += msg_chunk.T @ S_chunk into PSUM, per relation r.
Since (m * rel_r) @ W == m @ (diag(rel_r) @ W), the per-edge gating by
rel_emb folds into 6 pre-scaled weight matrices W_r, so messages never
need a per-edge elementwise multiply:
    h.T = relu(W.T @ x.T + sum_r W_r.T @ aggT_r)
Matmuls use float32r (fp32 replication mode, ~1.5e-4 rel rounding) with
a 256-wide moving dim for 1 cycle/row.

The same compiled program runs twice: layer 1 from x, layer 2 from the
host-assembled h1 table. Edges are identical across layers, so the chunk
structure (compile-time constant) is shared.
"""

import numpy as np

# ---- problem constants (fixed by the harness contract) ----
N_NODES = 50000
N_EDGES = 640000
D = 128
N_REL = 6
NCORES = 8
P = 128

OWN = N_NODES // NCORES  # 6250 real nodes per core
BLK = 256                # node block width (matmul moving dim; >=256 for f32r)
SHARD = 6400             # padded shard rows; SHARD % BLK == 0
NBLK = SHARD // BLK      # 25


def _build_layer_nc(chunksA, chunksB, *, d=D, n_rel=N_REL, shard=SHARD,
                    blk=BLK, ncores=NCORES):
    """Build the single-layer Bass program.

    chunksA/chunksB: [nblk][n_rel] 128-edge chunk counts per (block, rel)
    for source rows in the low/high table half — identical across cores.
    """
    import concourse.bass as bass
    import concourse.bacc as bacc
    import concourse.mybir as mybir
    import concourse.tile as tile

    f32 = mybir.dt.float32
    f32r = mybir.dt.float32r
    i16 = mybir.dt.int16
    Act = mybir.ActivationFunctionType
    Alu = mybir.AluOpType

    nblk = shard // blk
    nh = blk // P
    half_rows = ncores * shard // 2
    assert half_rows < 32768
    chunksA = [[int(c) for c in row] for row in chunksA]
    chunksB = [[int(c) for c in row] for row in chunksB]
    GA = sum(sum(r) for r in chunksA)
    GB = sum(sum(r) for r in chunksB)
    G = GA + GB
    maxcbA = max(sum(r) for r in chunksA)
    maxcbB = max(sum(r) for r in chunksB)

    nc = bacc.Bacc("TRN2", target_bir_lowering=False, debug=False,
                   num_devices=ncores)
    xtabA = nc.dram_tensor("xtabA", [half_rows, d], f32r, kind="ExternalInput")
    xtabB = nc.dram_tensor("xtabB", [half_rows, d], f32r, kind="ExternalInput")
    xown = nc.dram_tensor("xown", [shard, d], f32, kind="ExternalInput")
    wm = nc.dram_tensor("wm", [n_rel + 1, d, d], f32r, kind="ExternalInput")
    gidxA = nc.dram_tensor("gidxA", [P, max(GA, 1) * 8], i16,
                           kind="ExternalInput")
    gidxB = nc.dram_tensor("gidxB", [P, max(GB, 1) * 8], i16,
                           kind="ExternalInput")
    drel = nc.dram_tensor("drel", [P, G], f32, kind="ExternalInput")
    wts = nc.dram_tensor("wts", [P, G], f32, kind="ExternalInput")
    iota = nc.dram_tensor("iota", [P, blk], f32, kind="ExternalInput")
    ident = nc.dram_tensor("ident", [P, P], f32, kind="ExternalInput")
    hout = nc.dram_tensor("hout", [shard, d], f32, kind="ExternalOutput")

    with tile.TileContext(nc) as tc:
        with (
            tc.tile_pool(name="const", bufs=1) as cpool,
            tc.tile_pool(name="gatherA", bufs=4) as gpoolA,
            tc.tile_pool(name="gatherB", bufs=4) as gpoolB,
            tc.tile_pool(name="meta", bufs=3) as mpool,
            tc.tile_pool(name="onehot", bufs=8) as spool,
            tc.tile_pool(name="agg", bufs=2) as apool,
            tc.tile_pool(name="hwork", bufs=2) as hpool,
            tc.tile_pool(name="psA", bufs=3, space="PSUM") as psA,
            tc.tile_pool(name="psH", bufs=2, space="PSUM") as psH,
            tc.tile_pool(name="psT", bufs=2, space="PSUM") as psT,
        ):
            iot = cpool.tile([P, blk], f32, tag="iota")
            nc.sync.dma_start(iot[:], iota[:])
            idn = cpool.tile([P, P], f32, tag="ident")
            nc.sync.dma_start(idn[:], ident[:])
            wmt = cpool.tile([P, (n_rel + 1) * d], f32r, tag="wm")
            for j in range(n_rel + 1):
                nc.sync.dma_start(wmt[:, j * d:(j + 1) * d], wm[j])

            col = 0        # global meta column (emission order)
            colA = 0       # global A-region gather column
            colB = 0
            for b in range(nblk):
                cbA = sum(chunksA[b])
                cbB = sum(chunksB[b])
                cb = cbA + cbB
                # gathered source rows for this block, per table half
                if cbA:
                    xgA = gpoolA.tile([P, maxcbA, d], f32r, tag="xgA")
                    ixA = mpool.tile([P, max(maxcbA, 1) * 8], i16, tag="ixA")
                    nc.sync.dma_start(
                        ixA[:, :cbA * 8], gidxA[:, colA * 8:(colA + cbA) * 8])
                    for g0 in range(0, cbA, 8):
                        gn = min(8, cbA - g0)
                        nc.gpsimd.dma_gather(
                            out_ap=xgA[:, g0:g0 + gn, :], in_ap=xtabA[:],
                            idxs_ap=ixA[:, g0 * 8:(g0 + gn) * 8],
                            num_idxs=gn * P, num_idxs_reg=gn * P,
                            elem_size=d)
                if cbB:
                    xgB = gpoolB.tile([P, maxcbB, d], f32r, tag="xgB")
                    ixB = mpool.tile([P, max(maxcbB, 1) * 8], i16, tag="ixB")
                    nc.sync.dma_start(
                        ixB[:, :cbB * 8], gidxB[:, colB * 8:(colB + cbB) * 8])
                    for g0 in range(0, cbB, 8):
                        gn = min(8, cbB - g0)
                        nc.gpsimd.dma_gather(
                            out_ap=xgB[:, g0:g0 + gn, :], in_ap=xtabB[:],
                            idxs_ap=ixB[:, g0 * 8:(g0 + gn) * 8],
                            num_idxs=gn * P, num_idxs_reg=gn * P,
                            elem_size=d)
                dr = mpool.tile([P, max(cb, 1)], f32, tag="dr")
                wt = mpool.tile([P, max(cb, 1)], f32, tag="wt")
                nc.sync.dma_start(dr[:, :cb], drel[:, col:col + cb])
                nc.sync.dma_start(wt[:, :cb], wts[:, col:col + cb])

                aggT = apool.tile([P, n_rel, blk], f32r, tag="aggT")
                ci = 0      # block-local meta column, emission order
                ciA = 0     # block-local gather columns
                ciB = 0
                for r in range(n_rel):
                    nA, nB = chunksA[b][r], chunksB[b][r]
                    ntot = nA + nB
                    ps = psA.tile([P, blk], f32, tag="psA")
                    for c in range(ntot):
                        if c < nA:
                            lhs = xgA[:, ciA, :]
                            ciA += 1
                        else:
                            lhs = xgB[:, ciB, :]
                            ciB += 1
                        S = spool.tile([P, blk], f32r, tag="S")
                        nc.vector.tensor_scalar(
                            out=S[:], in0=iot[:],
                            scalar1=dr[:, ci:ci + 1],
                            scalar2=wt[:, ci:ci + 1],
                            op0=Alu.is_equal, op1=Alu.mult,
                        )
                        nc.tensor.matmul(
                            ps[:], lhsT=lhs, rhs=S[:],
                            start=(c == 0), stop=(c == ntot - 1),
                        )
                        ci += 1
                    nc.scalar.activation(aggT[:, r, :], ps[:], Act.Copy)

                # dense phase: h.T = relu(W.T x.T + sum_r W_r.T aggT_r)
                xr = hpool.tile([P, nh, d], f32, tag="xr")
                for h in range(nh):
                    nc.sync.dma_start(
                        xr[:, h, :],
                        xown[b * blk + h * P: b * blk + (h + 1) * P, :])
                pxT = psT.tile([P, blk], f32, tag="psT")
                for h in range(nh):
                    nc.tensor.transpose(
                        pxT[:, h * P:(h + 1) * P], xr[:, h, :], idn[:])
                xT = hpool.tile([P, blk], f32r, tag="xT")
                nc.scalar.activation(xT[:], pxT[:], Act.Copy)
                ph = psH.tile([P, blk], f32, tag="psH")
                nc.tensor.matmul(
                    ph[:], lhsT=wmt[:, 0:d],
                    rhs=xT[:], start=True, stop=False)
                for r in range(n_rel):
                    nc.tensor.matmul(
                        ph[:],
                        lhsT=wmt[:, (1 + r) * d:(2 + r) * d],
                        rhs=aggT[:, r, :],
                        start=False, stop=(r == n_rel - 1))
                hT = hpool.tile([P, blk], f32, tag="hT")
                nc.scalar.activation(hT[:], ph[:], Act.Relu)
                for h in range(nh):
                    pr = psT.tile([P, P], f32, tag="psT")
                    nc.tensor.transpose(
                        pr[:], hT[:, h * P:(h + 1) * P], idn[:])
                    hr = hpool.tile([P, P], f32, tag="hr")
                    nc.scalar.activation(hr[:], pr[:], Act.Copy)
                    nc.sync.dma_start(
                        hout[b * blk + h * P: b * blk + (h + 1) * P, :],
                        hr[:])
                col += cb
                colA += cbA
                colB += cbB
    nc.compile()
    return nc


def _preprocess(edge_index, edge_attr, *, n_nodes=N_NODES, n_rel=N_REL,
                own=OWN, shard=SHARD, blk=BLK, ncores=NCORES):
    """Index-only host preprocessing.

    Edges are assigned to (core, block, rel, half) cells (half = which
    25600-row table half the remapped source row falls in), padded to
    whole 128-edge chunks with per-cell counts equalized across cores.

    Returns (chunksA, chunksB, gidxA16, gidxB16, drelT, wT):
      gidx*16: [ncores, 128, G* * 8] int16 in DMAGatherAnt wrap layout
               (16-partition wrap, replicated 8x down the partitions).
      drelT/wT: [ncores, 128, G] with edge (p, g) = linear g*128 + p in
               emission order (per block: r0[A then B], r1[A then B]...).
    """
    ei = np.asarray(edge_index)
    src = ei[0].astype(np.int64)
    dst = ei[1].astype(np.int64)
    attr = np.asarray(edge_attr).astype(np.int64)
    nblk = shard // blk
    half_rows = ncores * shard // 2

    deg = np.bincount(dst, minlength=n_nodes)
    w_e = (1.0 / np.maximum(deg, 1.0)).astype(np.float32)[dst]
    rsrc = ((src // own) * shard + (src % own)).astype(np.int64)
    half = (rsrc >= half_rows).astype(np.int64)
    core = dst // own
    dloc = dst - core * own
    b_e = dloc // blk
    drel_e = (dloc - b_e * blk).astype(np.float32)

    # cell id ordered (core, block, rel, half)
    gid = (((core * nblk) + b_e) * n_rel + attr) * 2 + half
    order = np.argsort(gid, kind="stable")
    ncell = nblk * n_rel * 2
    counts = np.bincount(gid, minlength=ncores * ncell).reshape(ncores, ncell)
    chunks_flat = (-(-counts // P)).max(axis=0)  # [ncell], per-cell max
    cA = chunks_flat[0::2].reshape(nblk, n_rel).copy()
    cB = chunks_flat[1::2].reshape(nblk, n_rel).copy()
    empty = (cA + cB) == 0
    cA[empty] = 1
    chunks_flat = np.stack([cA.reshape(-1), cB.reshape(-1)], axis=1).reshape(-1)

    GA = int(cA.sum())
    GB = int(cB.sum())
    G = GA + GB

    # meta layout (emission order): per block, r0A,r0B,r1A,r1B,...
    meta_off = np.zeros(ncell + 1, np.int64)
    meta_off[1:] = np.cumsum(chunks_flat) * P
    # gather layouts (region order): per block, all A cells r0..r5 then next
    # block; likewise for B. Compute per-cell offsets into A/B index arrays.
    offA = np.zeros((nblk, n_rel), np.int64)
    offB = np.zeros((nblk, n_rel), np.int64)
    accA = accB = 0
    for b in range(nblk):
        for r in range(n_rel):
            offA[b, r] = accA
            accA += cA[b, r]
            offB[b, r] = accB
            accB += cB[b, r]

    gidxA16 = np.zeros((ncores, GA * P), np.int16)
    gidxB16 = np.zeros((ncores, GB * P), np.int16)
    drelA = np.zeros((ncores, G * P), np.float32)
    wA = np.zeros((ncores, G * P), np.float32)

    srt_rsrc = rsrc[order]
    srt_drel = drel_e[order]
    srt_w = w_e[order]
    gstart = np.zeros(ncores * ncell + 1, np.int64)
    gstart[1:] = np.cumsum(counts.reshape(-1))
    for k in range(ncores):
        for cell in range(ncell):
            g = k * ncell + cell
            s, e = gstart[g], gstart[g + 1]
            n = e - s
            if n == 0:
                continue
            o = meta_off[cell]
            drelA[k, o:o + n] = srt_drel[s:e]
            wA[k, o:o + n] = srt_w[s:e]
            blockrel = cell // 2
            b, r = blockrel // n_rel, blockrel % n_rel
            if cell % 2 == 0:
                oi = offA[b, r] * P
                gidxA16[k, oi:oi + n] = srt_rsrc[s:e].astype(np.int16)
            else:
                oi = offB[b, r] * P
                gidxB16[k, oi:oi + n] = (srt_rsrc[s:e]
                                         - half_rows).astype(np.int16)

    # meta arrays to [P, G]: edge (p, g) = linear g*P + p
    drelT = np.ascontiguousarray(drelA.reshape(ncores, G, P).transpose(0, 2, 1))
    wT = np.ascontiguousarray(wA.reshape(ncores, G, P).transpose(0, 2, 1))

    # index arrays to DMAGatherAnt wrap: [16, num/16] with
    # idx16[p, s] = idx[s*16 + p], then replicated 8x to 128 partitions.
    def wrap(a, Gn):
        if Gn == 0:
            return np.zeros((ncores, P, 8), np.int16)
        a = a.reshape(ncores, Gn * P // 16, 16).transpose(0, 2, 1)
        return np.ascontiguousarray(np.tile(a, (1, 8, 1)))

    return (cA.tolist(), cB.tolist(), wrap(gidxA16, GA), wrap(gidxB16, GB),
            drelT, wT)


_COMPILED = {}


def _get_nc(chunksA, chunksB):
    key = (tuple(tuple(r) for r in chunksA), tuple(tuple(r) for r in chunksB))
    if key not in _COMPILED:
        _COMPILED[key] = _build_layer_nc(chunksA, chunksB)
    return _COMPILED[key]


def _wmats(W, rel_emb):
    # W_r = diag(rel_r) @ W so that (m * rel_r) @ W == m @ W_r
    return np.ascontiguousarray(
        np.concatenate([W[None], rel_emb[:, :, None] * W[None]], axis=0)
    ).astype(np.float32)


def kernel(x, rel_emb, W1, W2, edge_index, edge_attr, _trace=False):
    from concourse.bass_utils import run_bass_kernel_spmd

    x = np.ascontiguousarray(np.asarray(x, np.float32))
    rel_emb = np.ascontiguousarray(np.asarray(rel_emb, np.float32))
    W1 = np.ascontiguousarray(np.asarray(W1, np.float32))
    W2 = np.ascontiguousarray(np.asarray(W2, np.float32))

    chunksA, chunksB, gA16, gB16, drelT, wT = _preprocess(
        edge_index, edge_attr)
    nc = _get_nc(chunksA, chunksB)
    iota = np.ascontiguousarray(
        np.broadcast_to(np.arange(BLK, dtype=np.float32), (P, BLK)))
    ident = np.eye(P, dtype=np.float32)
    half_rows = NCORES * SHARD // 2

    results = []

    def run_layer(xtab, wmats):
        xtab_v = xtab.reshape(NCORES, SHARD, D)
        in_maps = [
            dict(xtabA=xtab[:half_rows], xtabB=xtab[half_rows:],
                 xown=np.ascontiguousarray(xtab_v[k]), wm=wmats,
                 gidxA=gA16[k], gidxB=gB16[k], drel=drelT[k], wts=wT[k],
                 iota=iota, ident=ident)
            for k in range(NCORES)
        ]
        res = run_bass_kernel_spmd(
            nc, in_maps, core_ids=list(range(NCORES)), trace=False)
        results.append(res)
        return np.concatenate([r["hout"] for r in res.results], axis=0)

    wm1 = _wmats(W1, rel_emb)
    wm2 = _wmats(W2, rel_emb)
    xtab1 = np.zeros((NCORES * SHARD, D), np.float32)
    xtab1.reshape(NCORES, SHARD, D)[:, :OWN] = x.reshape(NCORES, OWN, D)
    h1 = run_layer(xtab1, wm1)
    h2 = run_layer(np.ascontiguousarray(h1), wm2)
    out = np.ascontiguousarray(
        h2.reshape(NCORES, SHARD, D)[:, :OWN].reshape(N_NODES, D))
    if _trace:
        kernel._last_results = results
    return out
